# revision 3
# baseline (speedup 1.0000x reference)
# Bass/Trainium2 kernel for nn_LoRARouter (topk_masking).
#
# Reference computes:
#   gated  = pooled @ Wg^T            [B, D]   (B=8192, D=4096)
#   logits = gated  @ Wr^T            [B, 7]
#   probs  = softmax(logits)
#   ranks  = argsort(argsort(-rand_noise))    per [7, B, :8] group
#   out[m,b,e] = probs[b,m] > 0.5 ? (rank<2)/2 : (rank<1)/1
#
# `gated` is only consumed by the second matmul, so
#   logits = pooled @ (Wr @ Wg)^T
# removing the 275-GFLOP [B,D]x[D,D] matmul. The problem is then at the
# HBM/PE ridge: read pooled (134 MB) + Wg (67 MB once across the fleet).
#
# Design (vs the 154 us AllGather version, where a ~60 us collective
# latency stalled the PE from 46->108 us):
#  - Wg is ROW-sharded (512 contraction rows per core). Each core
#    computes a PARTIAL WeffT = (Wr[:,rows] @ Wg[rows,:])^T for a d-block
#    as soon as the corresponding Wg piece lands, so the combine
#    collective can fire early.
#  - The combine is 2 pipelined 57-KB AllReduces (d-chunks 0-15, 16-31),
#    triggered at ~19/~30 us. AR output layout == input layout ==
#    matmul-ready d-major [128, 112] f32 on every core: one clean
#    load-back DMA, no descriptor spray (the AllGather design needed a
#    1024-descriptor rearrange that also blocked the x stream via shared
#    DMA sem lanes).
#  - All matmuls are fp16 two-limb (hi + 2^11-scaled lo) with fp32 PSUM:
#    logits = A + B/2048. 1 cycle/col on the PE; per-elem err ~2^-22 vs
#    min decision margin 5.5e-5.
#  - x streams as 16 1-MB single-use pieces (batch-half-major, chunk
#    groups of 4), so logits PSUM chains are gated by x ARRIVAL, not by
#    the collective: chunk c needs AR half c//16, which lands long
#    before x piece c//4 + 8*bh.
#  - Select tail vectorized per batch-half; only bh1's half-tail is
#    exposed after the last matmul.
#
# Sharding (8 cores):
#  - pooled_hidden, rand_noise, output: batch-sharded (1024 rows/core)
#  - Wg: row-sharded (512 contraction rows/core); 2 AllReduces of WeffT.

import numpy as np

import concourse.bass as bass
import concourse.bacc as bacc
import concourse.mybir as mybir
import concourse.tile as tile
from concourse.bass_utils import run_bass_kernel_spmd

F32 = mybir.dt.float32
F16 = mybir.dt.float16
N_CORES = 8
B, D, NM, NE = 8192, 4096, 7, 8      # batch, d_model, n_modules, n_experts
BS = B // N_CORES                    # 1024 batch rows per core
SH = D // N_CORES                    # 512 Wg contraction rows per core
NK = D // 128                        # 32 contraction chunks of 128
NEC = SH // 128                      # 4 own e-chunks
NBC = BS // 128                      # 8 batch chunks of 128 per core
GRP = NM * NE                        # 56 columns per batch chunk
W = NBC * GRP                        # 448 free columns in [128, 448] tiles
SCALE = 2048.0                       # lo-limb scale 2^11
WPC = 4                              # wg pieces (1024 d-cols each, 2 MB)
XPC = 16                             # x pieces (4 chunks x 1 batch-half, 1 MB)
NQ = 2                               # AllReduce halves (16 d-chunks each)

ALU = mybir.AluOpType
AF = mybir.ActivationFunctionType

_CACHE = {}
LAST_RESULTS = None  # test harness introspection


def _build_program():
    nc = bacc.Bacc(
        "TRN2", target_bir_lowering=False, debug=False, num_devices=N_CORES
    )

    # x limbs: piece P = 8*bh + p holds chunks 4p..4p+3, batch cols
    # [512bh, 512bh+512); row layout [cl(4)][limb(2)][512 batch]
    xi = nc.dram_tensor("xi", [XPC * 128, 4096], F16, kind="ExternalInput")
    # wg limbs: piece p holds d-cols [1024p, 1024p+1024), own 512 e-rows;
    # row layout [ec(4)][limb(2)][1024 d]
    wgi = nc.dram_tensor("wgi", [WPC * 128, 8192], F16, kind="ExternalInput")
    # Wr limbs, e-major: wr*[p, 7c+m] = Wr[m, 512i + 128c + p]
    wrh = nc.dram_tensor("wrh", [128, NEC * NM], F16, kind="ExternalInput")
    wrl = nc.dram_tensor("wrl", [128, NEC * NM], F16, kind="ExternalInput")
    nzin = nc.dram_tensor("nz", [128, W], F32, kind="ExternalInput")
    cstin = nc.dram_tensor("cst", [128, W], F32, kind="ExternalInput")
    o = nc.dram_tensor("o", [128, W], F32, kind="ExternalOutput")

    # AllReduce bounce buffers: half q carries d-chunks 16q..16q+15 of the
    # partial WeffT in final d-major layout [p, 7*lc + m] (f32).
    wq_in = [nc.dram_tensor(f"wqin{q}", [128, 16 * NM], F32) for q in range(NQ)]
    wq_out = [
        nc.dram_tensor(f"wqout{q}", [128, 16 * NM], F32, addr_space="Shared")
        for q in range(NQ)
    ]

    with tile.TileContext(nc) as tc:
        with (
            tc.tile_pool(name="wgp", bufs=2) as wgp,
            tc.tile_pool(name="xp", bufs=XPC) as xp,
            tc.tile_pool(name="sp", bufs=1) as sp,
            tc.tile_pool(name="scr", bufs=2) as scp,
            tc.tile_pool(name="sm", bufs=16) as smp,
            tc.tile_pool(name="pw", bufs=2, space="PSUM") as pw,
            tc.tile_pool(name="pl", bufs=4, space="PSUM") as pl,
            tc.tile_pool(name="pt", bufs=2, space="PSUM") as pt,
        ):
            # ---- small input DMAs on the scalar HWDGE ring ----
            wrh_sb = sp.tile([128, NEC * NM], F16, tag="wrh")
            wrl_sb = sp.tile([128, NEC * NM], F16, tag="wrl")
            nz = sp.tile([128, W], F32, tag="nz")
            cstt = sp.tile([128, W], F32, tag="cst")
            nc.scalar.dma_start(wrh_sb[:], wrh[:])
            nc.scalar.dma_start(wrl_sb[:], wrl[:])
            nc.scalar.dma_start(nz[:], nzin[:])
            nc.scalar.dma_start(cstt[:], cstin[:])
            # warm the Exp activation table while the ring is idle
            warm = smp.tile([128, 1], F32, tag="warm")
            nc.scalar.activation(warm[:], nz[:, :1], AF.Exp)

            # identity for PE transposes (only [:7,:7] used)
            ident = sp.tile([128, 128], F32, tag="ident")
            from concourse.masks import make_identity
            make_identity(nc, ident[:])

            # ---- big input streams on the sync HWDGE ring (wg first) ----
            wgt = []
            for p in range(WPC):
                t = wgp.tile([128, 8192], F16, tag="wg")
                nc.sync.dma_start(t[:], wgi[p * 128:(p + 1) * 128, :])
                wgt.append(t)
            xts = []
            for P in range(XPC):
                t = xp.tile([128, 4096], F16, tag="x")
                nc.sync.dma_start(t[:], xi[P * 128:(P + 1) * 128, :])
                xts.append(t)

            # ---- partial WeffT per d-block, AR halves triggered early ----
            wstage = [
                sp.tile([128, 16 * NM], F32, tag=f"wstage{q}", name=f"wstage{q}")
                for q in range(NQ)
            ]
            for p in range(WPC):
                for j in (0, 1):
                    dj = 2 * p + j          # d-block of 512 (4 d-chunks)
                    psA = pw.tile([NM, 512], F32, tag="pw", name=f"wA{dj}")
                    psB = pw.tile([NM, 512], F32, tag="pw", name=f"wB{dj}")
                    for ec in range(NEC):
                        ghi = wgt[p][:, ec * 2048 + j * 512:
                                     ec * 2048 + j * 512 + 512]
                        glo = wgt[p][:, ec * 2048 + 1024 + j * 512:
                                     ec * 2048 + 1024 + j * 512 + 512]
                        rsl = slice(ec * NM, (ec + 1) * NM)
                        st, sp_ = (ec == 0), (ec == NEC - 1)
                        nc.tensor.matmul(psA[:], wrh_sb[:, rsl], ghi,
                                         start=st, stop=sp_)
                        nc.tensor.matmul(psB[:], wrl_sb[:, rsl], ghi,
                                         start=st, stop=False)
                        nc.tensor.matmul(psB[:], wrh_sb[:, rsl], glo,
                                         start=False, stop=sp_)
                    # combine wf = psA + psB/SCALE  [7, 512] f32
                    tb = scp.tile([NM, 512], F32, tag="scr2")
                    wf = scp.tile([NM, 512], F32, tag="scr3")
                    nc.vector.tensor_scalar_mul(tb[:], psB[:], 1.0 / SCALE)
                    nc.vector.tensor_tensor(wf[:], tb[:], psA[:], ALU.add)
                    # transpose the 4 d-chunks into the AR staging tile
                    q = dj // 4
                    for t_ in range(4):
                        c = 4 * dj + t_                  # global d-chunk
                        lc = c - 16 * q
                        tr = pt.tile([128, NM], F32, tag="pt")
                        nc.tensor.transpose(
                            tr[:], wf[:, t_ * 128:(t_ + 1) * 128], ident[:7, :7]
                        )
                        nc.vector.tensor_copy(
                            wstage[q][:, lc * NM:(lc + 1) * NM], tr[:]
                        )
                if p % 2 == 1:
                    q = p // 2
                    nc.scalar.dma_start(wq_in[q][:], wstage[q][:])
                    nc.gpsimd.collective_compute(
                        "AllReduce",
                        ALU.add,
                        replica_groups=[list(range(N_CORES))],
                        ins=[wq_in[q][:]],
                        outs=[wq_out[q][:]],
                    )

            # ---- load back the reduced WeffT halves, split fp16 limbs ----
            whiT, wloT = [], []
            for q in range(NQ):
                wga = sp.tile([128, 16 * NM], F32, tag=f"wga{q}")
                nc.scalar.dma_start(wga[:], wq_out[q][:])
                whi = sp.tile([128, 16 * NM], F16, tag=f"whiT{q}")
                wlo = sp.tile([128, 16 * NM], F16, tag=f"wloT{q}")
                hi32 = scp.tile([128, 16 * NM], F32, tag="scr4")
                dif = scp.tile([128, 16 * NM], F32, tag="scr4")
                nc.vector.tensor_copy(whi[:], wga[:])
                nc.vector.tensor_copy(hi32[:], whi[:])
                nc.vector.tensor_tensor(dif[:], wga[:], hi32[:], ALU.subtract)
                nc.vector.tensor_scalar_mul(wlo[:], dif[:], SCALE)
                whiT.append(whi)
                wloT.append(wlo)

            # ---- expert ranks from rand_noise (overlaps the DMA phase on
            # DVE). r[e] = #{j<e: v_j >= v_e} + #{j>e: v_j > v_e}; acc
            # starts at cst[e] = 7-e; each offset's comparison adds 1 at
            # the A position and subtracts 1 at the B position. ----
            acc = sp.tile([128, W], F32, tag="acc")
            nc.vector.tensor_copy(acc[:], cstt[:])
            nz_r = nz[:].rearrange("p (c m e) -> p c m e", m=NM, e=NE)
            acc_r = acc[:].rearrange("p (c m e) -> p c m e", m=NM, e=NE)
            for off in range(1, NE):
                wdt = NE - off
                scr = scp.tile([128, NBC * NM * 7], F32, tag="scr")
                scr_v = scr[:, : NBC * NM * wdt].rearrange(
                    "p (c m e) -> p c m e", m=NM, e=wdt
                )
                nc.vector.tensor_tensor(
                    scr_v, nz_r[:, :, :, 0:wdt], nz_r[:, :, :, off:NE], ALU.is_ge
                )
                nc.vector.tensor_tensor(
                    acc_r[:, :, :, off:NE], acc_r[:, :, :, off:NE], scr_v, ALU.add
                )
                nc.vector.tensor_tensor(
                    acc_r[:, :, :, 0:wdt], acc_r[:, :, :, 0:wdt], scr_v, ALU.subtract
                )

            # ---- logitsT: two-limb chains per batch half over 32 chunks ----
            psA = [pl.tile([NM, 512], F32, tag="pl", name=f"pA{bh}") for bh in (0, 1)]
            psB = [pl.tile([NM, 512], F32, tag="pl", name=f"pB{bh}") for bh in (0, 1)]

            logT = sp.tile([NM, BS], F32, tag="logT")
            thr = sp.tile([128, NBC * NM], F32, tag="thr")
            threp = sp.tile([128, W], F32, tag="threp")
            valrep = sp.tile([128, W], F32, tag="valrep")
            outt = sp.tile([128, W], F32, tag="outt")
            thr_v = thr[:].rearrange("p (c m) -> p c m", m=NM)
            threp_v = threp[:].rearrange("p (c m e) -> p c m e", m=NM, e=NE)
            HBC = NBC // 2          # 4 batch chunks per half
            HW_ = W // 2            # 224 select columns per half

            def mm_block(bh):
                for c in range(NK):
                    q, lc = c // 16, c % 16
                    whi = whiT[q][:, lc * NM:(lc + 1) * NM]
                    wlo = wloT[q][:, lc * NM:(lc + 1) * NM]
                    xt = xts[8 * bh + c // 4]
                    base = (c % 4) * 1024
                    xhi = xt[:, base:base + 512]
                    xlo = xt[:, base + 512:base + 1024]
                    st, sp_ = (c == 0), (c == NK - 1)
                    nc.tensor.matmul(psA[bh][:], whi, xhi, start=st, stop=sp_)
                    nc.tensor.matmul(psB[bh][:], wlo, xhi, start=st, stop=False)
                    nc.tensor.matmul(psB[bh][:], whi, xlo, start=False, stop=sp_)

            def tail_half(bh):
                # combine: logT half = A + B/SCALE
                tb = scp.tile([NM, 512], F32, tag="scr5")
                nc.vector.tensor_scalar_mul(tb[:], psB[bh][:], 1.0 / SCALE)
                nc.vector.tensor_tensor(
                    logT[:, bh * 512:(bh + 1) * 512], tb[:], psA[bh][:], ALU.add
                )
                # softmax>0.5 -> thr in {1,2} for this half's batch chunks
                for bc in range(bh * HBC, (bh + 1) * HBC):
                    plt = pt.tile([128, NM], F32, tag="pt")
                    nc.tensor.transpose(
                        plt[:], logT[:, bc * 128:(bc + 1) * 128], ident[:7, :7]
                    )
                    # |logits| < 9: exp cannot overflow fp32, skip max-sub
                    ssum = smp.tile([128, 1], F32, tag="ssum")
                    shalf = smp.tile([128, 1], F32, tag="shalf")
                    expt = smp.tile([128, NM], F32, tag="expt")
                    nc.scalar.activation(
                        expt[:], plt[:], AF.Exp, accum_out=ssum[:]
                    )
                    nc.vector.tensor_scalar_mul(shalf[:], ssum[:], 0.5)
                    nc.vector.tensor_scalar(
                        out=thr[:, bc * NM:(bc + 1) * NM], in0=expt[:],
                        scalar1=shalf[:], scalar2=1.0, op0=ALU.is_gt, op1=ALU.add,
                    )
                # select for this half (all on DVE; gpsimd runs these
                # strided ops 4-5x slower)
                cs = slice(bh * HBC, (bh + 1) * HBC)
                ws = slice(bh * HW_, (bh + 1) * HW_)
                for e in range(NE):
                    nc.vector.tensor_copy(threp_v[:, cs, :, e], thr_v[:, cs, :])
                nc.vector.tensor_scalar(
                    out=valrep[:, ws], in0=threp[:, ws], scalar1=-0.5,
                    scalar2=1.5, op0=ALU.mult, op1=ALU.add,
                )
                nc.vector.tensor_tensor(
                    outt[:, ws], acc[:, ws], threp[:, ws], ALU.is_lt
                )
                nc.vector.tensor_tensor(
                    outt[:, ws], outt[:, ws], valrep[:, ws], ALU.mult
                )

            mm_block(0)
            tail_half(0)
            mm_block(1)
            tail_half(1)
            nc.scalar.dma_start(o[:], outt[:])

    nc.compile()
    return nc


def _get_program():
    if "nc" not in _CACHE:
        _CACHE["nc"] = _build_program()
    return _CACHE["nc"]


def _split16(x):
    hi = x.astype(np.float16)
    lo = ((x - hi.astype(np.float32)) * SCALE).astype(np.float16)
    return hi, lo


def _const_input():
    base = (7.0 - np.arange(NE, dtype=np.float32))
    return np.ascontiguousarray(
        np.broadcast_to(np.tile(base, NBC * NM), (128, W))
    )


def _prep_core(i, ph, wg_full, rn):
    bsl = slice(i * BS, (i + 1) * BS)
    xc = np.ascontiguousarray(ph[bsl].T).reshape(NK, 128, BS)  # [32,128,1024]
    hi, lo = _split16(xc)
    # piece P = 8*bh + p: [128, cl(4) x limb(2) x 512]
    pieces = []
    for bh in (0, 1):
        hs = hi[:, :, bh * 512:(bh + 1) * 512]    # [32,128,512]
        ls = lo[:, :, bh * 512:(bh + 1) * 512]
        # [32,2,128,512] -> per piece p: chunks 4p..4p+3
        both = np.stack([hs, ls], axis=1)         # [32,2,128,512]
        arr = both.reshape(8, 4, 2, 128, 512).transpose(0, 3, 1, 2, 4)
        pieces.append(arr.reshape(8 * 128, 4096))
    xi = np.ascontiguousarray(np.concatenate(pieces, axis=0))  # [2048, 4096]

    esl = slice(i * SH, (i + 1) * SH)
    wgs = wg_full[esl]                            # [512, 4096] own rows
    wps = []
    for p in range(WPC):
        blk = wgs[:, p * 1024:(p + 1) * 1024]     # [512, 1024]
        bhi, blo = _split16(blk.reshape(NEC, 128, 1024))
        both = np.stack([bhi, blo], axis=1)       # [4,2,128,1024]
        wps.append(both.transpose(2, 0, 1, 3).reshape(128, 8192))
    wgi = np.ascontiguousarray(np.concatenate(wps, axis=0))    # [512, 8192]

    # nz[p, c*56 + m*8 + e] = rn[m, 1024*i + 128*c + p, e]
    nz_i = np.ascontiguousarray(
        rn[:, bsl, :].transpose(1, 0, 2)
        .reshape(NBC, 128, GRP).transpose(1, 0, 2).reshape(128, W)
    )

    # Wr slice for own rows: wrt[p, 7c+m] = Wr[m, 512i + 128c + p]
    wrt = np.ascontiguousarray(
        _WR_T[esl].reshape(NEC, 128, NM).transpose(1, 0, 2).reshape(128, NEC * NM)
    )
    wrh_i, wrl_i = _split16(wrt)
    return xi, wgi, nz_i, np.ascontiguousarray(wrh_i), np.ascontiguousarray(wrl_i)


_WR_T = None


def kernel(pooled_hidden, Wg, Wr, rand_noise):
    global LAST_RESULTS, _WR_T
    ph = np.asarray(pooled_hidden, dtype=np.float32)
    wg_full = np.asarray(Wg, dtype=np.float32)
    wr = np.asarray(Wr, dtype=np.float32)
    rn = np.asarray(rand_noise, dtype=np.float32)
    _WR_T = np.ascontiguousarray(wr.T)            # [4096, 7]

    nc = _get_program()
    cst = _const_input()

    in_maps = []
    for i in range(N_CORES):
        xi, wgi, nz_i, wrh_i, wrl_i = _prep_core(i, ph, wg_full, rn)
        in_maps.append(
            {"xi": xi, "wgi": wgi, "wrh": wrh_i, "wrl": wrl_i,
             "nz": nz_i, "cst": cst}
        )

    res = run_bass_kernel_spmd(nc, in_maps, list(range(N_CORES)))
    LAST_RESULTS = res

    out = np.empty((NM, B, NE), dtype=np.float32)
    for i, r in enumerate(res.results):
        oc = r["o"]  # [128, 448]
        out[:, i * BS:(i + 1) * BS, :] = (
            oc.reshape(128, NBC, NM, NE).transpose(2, 1, 0, 3).reshape(NM, BS, NE)
        )
    return out


# revision 7
# speedup vs baseline: 1.0837x; 1.0837x over previous
# Bass/Trainium2 kernel for nn_LoRARouter (topk_masking).
#
# Reference computes:
#   gated  = pooled @ Wg^T            [B, D]   (B=8192, D=4096)
#   logits = gated  @ Wr^T            [B, 7]
#   probs  = softmax(logits)
#   ranks  = argsort(argsort(-rand_noise))    per [7, B, :8] group
#   out[m,b,e] = probs[b,m] > 0.5 ? (rank<2)/2 : (rank<1)/1
#
# `gated` is only consumed by the second matmul, so
#   logits = pooled @ (Wr @ Wg)^T
# removing the 275-GFLOP [B,D]x[D,D] matmul. The problem is then at the
# HBM/PE ridge: read pooled (134 MB) + Wg (67 MB once across the fleet).
#
# Design (vs the 154 us AllGather version, where a ~60 us collective
# latency stalled the PE from 46->108 us):
#  - Wg is ROW-sharded (512 contraction rows per core). Each core
#    computes a PARTIAL WeffT = (Wr[:,rows] @ Wg[rows,:])^T for a d-block
#    as soon as the corresponding Wg piece lands, so the combine
#    collective can fire early.
#  - The combine is 2 pipelined 57-KB AllReduces (d-chunks 0-15, 16-31),
#    triggered at ~19/~30 us. AR output layout == input layout ==
#    matmul-ready d-major [128, 112] f32 on every core: one clean
#    load-back DMA, no descriptor spray (the AllGather design needed a
#    1024-descriptor rearrange that also blocked the x stream via shared
#    DMA sem lanes).
#  - All matmuls are fp16 two-limb (hi + 2^11-scaled lo) with fp32 PSUM:
#    logits = A + B/2048. 1 cycle/col on the PE; per-elem err ~2^-22 vs
#    min decision margin 5.5e-5.
#  - x streams as 16 1-MB single-use pieces (batch-half-major, chunk
#    groups of 4), so logits PSUM chains are gated by x ARRIVAL, not by
#    the collective: chunk c needs AR half c//16, which lands long
#    before x piece c//4 + 8*bh.
#  - Select tail vectorized per batch-half; only bh1's half-tail is
#    exposed after the last matmul.
#
# Sharding (8 cores):
#  - pooled_hidden, rand_noise, output: batch-sharded (1024 rows/core)
#  - Wg: row-sharded (512 contraction rows/core); 2 AllReduces of WeffT.

import numpy as np

import concourse.bass as bass
import concourse.bacc as bacc
import concourse.mybir as mybir
import concourse.tile as tile
from concourse.bass_utils import run_bass_kernel_spmd

F32 = mybir.dt.float32
F16 = mybir.dt.float16
N_CORES = 8
B, D, NM, NE = 8192, 4096, 7, 8      # batch, d_model, n_modules, n_experts
BS = B // N_CORES                    # 1024 batch rows per core
SH = D // N_CORES                    # 512 Wg contraction rows per core
NK = D // 128                        # 32 contraction chunks of 128
NEC = SH // 128                      # 4 own e-chunks
NBC = BS // 128                      # 8 batch chunks of 128 per core
GRP = NM * NE                        # 56 columns per batch chunk
W = NBC * GRP                        # 448 free columns in [128, 448] tiles
SCALE = 2048.0                       # lo-limb scale 2^11
WPC = 4                              # wg pieces (1024 d-cols each, 2 MB)
XPC = 16                             # x pieces (4 chunks x 1 batch-half, 1 MB)
NQ = 2                               # AllReduce halves (16 d-chunks each)

ALU = mybir.AluOpType
AF = mybir.ActivationFunctionType

_CACHE = {}
LAST_RESULTS = None  # test harness introspection


def _build_program():
    nc = bacc.Bacc(
        "TRN2", target_bir_lowering=False, debug=False, num_devices=N_CORES
    )

    # x limbs: piece P = 8*bh + p holds chunks 4p..4p+3, batch cols
    # [512bh, 512bh+512); row layout [cl(4)][limb(2)][512 batch]
    xi = nc.dram_tensor("xi", [XPC * 128, 4096], F16, kind="ExternalInput")
    # wg limbs: piece p holds d-cols [1024p, 1024p+1024), own 512 e-rows;
    # row layout [ec(4)][limb(2)][1024 d]
    wgi = nc.dram_tensor("wgi", [WPC * 128, 8192], F16, kind="ExternalInput")
    # Wr limbs, e-major: wr*[p, 7c+m] = Wr[m, 512i + 128c + p]
    wrh = nc.dram_tensor("wrh", [128, NEC * NM], F16, kind="ExternalInput")
    wrl = nc.dram_tensor("wrl", [128, NEC * NM], F16, kind="ExternalInput")
    nzin = nc.dram_tensor("nz", [128, W], F32, kind="ExternalInput")
    cstin = nc.dram_tensor("cst", [128, W], F32, kind="ExternalInput")
    o = nc.dram_tensor("o", [128, W], F32, kind="ExternalOutput")

    # AllReduce bounce buffers: half q carries d-chunks 16q..16q+15 of the
    # partial WeffT in final d-major layout [p, 7*lc + m] (f32).
    wq_in = [nc.dram_tensor(f"wqin{q}", [128, 16 * NM], F32) for q in range(NQ)]
    wq_out = [
        nc.dram_tensor(f"wqout{q}", [128, 16 * NM], F32, addr_space="Shared")
        for q in range(NQ)
    ]
    # warmup collective: absorbs ncfw first-call init + rank-start skew
    # under the DMA streaming phase, so the real ARs run at the ~11 us
    # benchmark latency instead of ~55 us.
    wu_in = nc.dram_tensor("wuin", [128, 4], F32)
    wu_out = nc.dram_tensor("wuout", [128, 4], F32, addr_space="Shared")

    with tile.TileContext(nc) as tc:
        with (
            tc.tile_pool(name="wgp", bufs=WPC) as wgp,
            tc.tile_pool(name="xp", bufs=12) as xp,
            tc.tile_pool(name="sp", bufs=1) as sp,
            tc.tile_pool(name="scr", bufs=2) as scp,
            tc.tile_pool(name="sm", bufs=16) as smp,
            tc.tile_pool(name="pw", bufs=2, space="PSUM") as pw,
            tc.tile_pool(name="pl", bufs=4, space="PSUM") as pl,
            tc.tile_pool(name="pt", bufs=2, space="PSUM") as pt,
        ):
            # warmup collective first: doorbell fires at program start
            nc.gpsimd.collective_compute(
                "AllReduce",
                ALU.add,
                replica_groups=[list(range(N_CORES))],
                ins=[wu_in[:]],
                outs=[wu_out[:]],
            )

            # ---- small input DMAs on the scalar HWDGE ring ----
            wrh_sb = sp.tile([128, NEC * NM], F16, tag="wrh")
            wrl_sb = sp.tile([128, NEC * NM], F16, tag="wrl")
            nz = sp.tile([128, W], F32, tag="nz")
            cstt = sp.tile([128, W], F32, tag="cst")
            nc.scalar.dma_start(wrh_sb[:], wrh[:])
            nc.scalar.dma_start(wrl_sb[:], wrl[:])
            nc.scalar.dma_start(nz[:], nzin[:])
            nc.scalar.dma_start(cstt[:], cstin[:])
            # warm the Exp activation table while the ring is idle
            warm = smp.tile([128, 1], F32, tag="warm")
            nc.scalar.activation(warm[:], nz[:, :1], AF.Exp)

            # identity for PE transposes (only [:7,:7] used)
            ident = sp.tile([128, 128], F32, tag="ident")
            from concourse.masks import make_identity
            make_identity(nc, ident[:])

            # ---- big input streams on the sync HWDGE ring (wg first) ----
            wgt = []
            for p in range(WPC):
                t = wgp.tile([128, 8192], F16, tag="wg")
                nc.sync.dma_start(t[:], wgi[p * 128:(p + 1) * 128, :])
                wgt.append(t)
            xts = []
            for P in range(XPC):
                t = xp.tile([128, 4096], F16, tag="x")
                nc.sync.dma_start(t[:], xi[P * 128:(P + 1) * 128, :])
                xts.append(t)

            # ---- partial WeffT per d-block, AR halves triggered early ----
            wstage = [
                sp.tile([128, 16 * NM], F32, tag=f"wstage{q}", name=f"wstage{q}")
                for q in range(NQ)
            ]
            for p in range(WPC):
                for j in (0, 1):
                    dj = 2 * p + j          # d-block of 512 (4 d-chunks)
                    psA = pw.tile([NM, 512], F32, tag="pw", name=f"wA{dj}")
                    psB = pw.tile([NM, 512], F32, tag="pw", name=f"wB{dj}")
                    for ec in range(NEC):
                        ghi = wgt[p][:, ec * 2048 + j * 512:
                                     ec * 2048 + j * 512 + 512]
                        glo = wgt[p][:, ec * 2048 + 1024 + j * 512:
                                     ec * 2048 + 1024 + j * 512 + 512]
                        rsl = slice(ec * NM, (ec + 1) * NM)
                        st, sp_ = (ec == 0), (ec == NEC - 1)
                        nc.tensor.matmul(psA[:], wrh_sb[:, rsl], ghi,
                                         start=st, stop=sp_)
                        nc.tensor.matmul(psB[:], wrl_sb[:, rsl], ghi,
                                         start=st, stop=False)
                        nc.tensor.matmul(psB[:], wrh_sb[:, rsl], glo,
                                         start=False, stop=sp_)
                    # combine wf = psA + psB/SCALE  [7, 512] f32
                    tb = scp.tile([NM, 512], F32, tag="scr2")
                    wf = scp.tile([NM, 512], F32, tag="scr3")
                    nc.vector.tensor_scalar_mul(tb[:], psB[:], 1.0 / SCALE)
                    nc.vector.tensor_tensor(wf[:], tb[:], psA[:], ALU.add)
                    # transpose the 4 d-chunks into the AR staging tile
                    q = dj // 4
                    for t_ in range(4):
                        c = 4 * dj + t_                  # global d-chunk
                        lc = c - 16 * q
                        tr = pt.tile([128, NM], F32, tag="pt")
                        nc.tensor.transpose(
                            tr[:], wf[:, t_ * 128:(t_ + 1) * 128], ident[:7, :7]
                        )
                        nc.vector.tensor_copy(
                            wstage[q][:, lc * NM:(lc + 1) * NM], tr[:]
                        )
                if p % 2 == 1:
                    q = p // 2
                    nc.scalar.dma_start(wq_in[q][:], wstage[q][:])
                    nc.gpsimd.collective_compute(
                        "AllReduce",
                        ALU.add,
                        replica_groups=[list(range(N_CORES))],
                        ins=[wq_in[q][:]],
                        outs=[wq_out[q][:]],
                    )

            # ---- load back the reduced WeffT halves, split fp16 limbs ----
            whiT, wloT = [], []
            for q in range(NQ):
                wga = sp.tile([128, 16 * NM], F32, tag=f"wga{q}")
                nc.scalar.dma_start(wga[:], wq_out[q][:])
                whi = sp.tile([128, 16 * NM], F16, tag=f"whiT{q}")
                wlo = sp.tile([128, 16 * NM], F16, tag=f"wloT{q}")
                hi32 = scp.tile([128, 16 * NM], F32, tag="scr4")
                dif = scp.tile([128, 16 * NM], F32, tag="scr4")
                nc.vector.tensor_copy(whi[:], wga[:])
                nc.vector.tensor_copy(hi32[:], whi[:])
                nc.vector.tensor_tensor(dif[:], wga[:], hi32[:], ALU.subtract)
                nc.vector.tensor_scalar_mul(wlo[:], dif[:], SCALE)
                whiT.append(whi)
                wloT.append(wlo)

            # ---- expert ranks from rand_noise (overlaps the DMA phase on
            # DVE). r[e] = #{j<e: v_j >= v_e} + #{j>e: v_j > v_e}; acc
            # starts at cst[e] = 7-e; each offset's comparison adds 1 at
            # the A position and subtracts 1 at the B position. ----
            acc = sp.tile([128, W], F32, tag="acc")
            nc.vector.tensor_copy(acc[:], cstt[:])
            nz_r = nz[:].rearrange("p (c m e) -> p c m e", m=NM, e=NE)
            acc_r = acc[:].rearrange("p (c m e) -> p c m e", m=NM, e=NE)
            for off in range(1, NE):
                wdt = NE - off
                scr = scp.tile([128, NBC * NM * 7], F32, tag="scr")
                scr_v = scr[:, : NBC * NM * wdt].rearrange(
                    "p (c m e) -> p c m e", m=NM, e=wdt
                )
                nc.vector.tensor_tensor(
                    scr_v, nz_r[:, :, :, 0:wdt], nz_r[:, :, :, off:NE], ALU.is_ge
                )
                nc.vector.tensor_tensor(
                    acc_r[:, :, :, off:NE], acc_r[:, :, :, off:NE], scr_v, ALU.add
                )
                nc.vector.tensor_tensor(
                    acc_r[:, :, :, 0:wdt], acc_r[:, :, :, 0:wdt], scr_v, ALU.subtract
                )

            # ---- logitsT: two-limb chains per batch half over 32 chunks ----
            psA = [pl.tile([NM, 512], F32, tag="pl", name=f"pA{bh}") for bh in (0, 1)]
            psB = [pl.tile([NM, 512], F32, tag="pl", name=f"pB{bh}") for bh in (0, 1)]

            logT = sp.tile([NM, BS], F32, tag="logT")
            thr = sp.tile([128, NBC * NM], F32, tag="thr")
            threp = sp.tile([128, W], F32, tag="threp")
            valrep = sp.tile([128, W], F32, tag="valrep")
            outt = sp.tile([128, W], F32, tag="outt")
            thr_v = thr[:].rearrange("p (c m) -> p c m", m=NM)
            threp_v = threp[:].rearrange("p (c m e) -> p c m e", m=NM, e=NE)
            HBC = NBC // 2          # 4 batch chunks per half
            HW_ = W // 2            # 224 select columns per half

            def mm_block(bh):
                for c in range(NK):
                    q, lc = c // 16, c % 16
                    whi = whiT[q][:, lc * NM:(lc + 1) * NM]
                    wlo = wloT[q][:, lc * NM:(lc + 1) * NM]
                    xt = xts[8 * bh + c // 4]
                    base = (c % 4) * 1024
                    xhi = xt[:, base:base + 512]
                    xlo = xt[:, base + 512:base + 1024]
                    st, sp_ = (c == 0), (c == NK - 1)
                    nc.tensor.matmul(psA[bh][:], whi, xhi, start=st, stop=sp_)
                    nc.tensor.matmul(psB[bh][:], wlo, xhi, start=st, stop=False)
                    nc.tensor.matmul(psB[bh][:], whi, xlo, start=False, stop=sp_)

            def tail_half(bh):
                # combine: logT half = A + B/SCALE
                tb = scp.tile([NM, 512], F32, tag="scr5")
                nc.vector.tensor_scalar_mul(tb[:], psB[bh][:], 1.0 / SCALE)
                nc.vector.tensor_tensor(
                    logT[:, bh * 512:(bh + 1) * 512], tb[:], psA[bh][:], ALU.add
                )
                # softmax>0.5 -> thr in {1,2} for this half's batch chunks
                for bc in range(bh * HBC, (bh + 1) * HBC):
                    plt = pt.tile([128, NM], F32, tag="pt")
                    nc.tensor.transpose(
                        plt[:], logT[:, bc * 128:(bc + 1) * 128], ident[:7, :7]
                    )
                    # |logits| < 9: exp cannot overflow fp32, skip max-sub
                    ssum = smp.tile([128, 1], F32, tag="ssum")
                    shalf = smp.tile([128, 1], F32, tag="shalf")
                    expt = smp.tile([128, NM], F32, tag="expt")
                    nc.scalar.activation(
                        expt[:], plt[:], AF.Exp, accum_out=ssum[:]
                    )
                    nc.vector.tensor_scalar_mul(shalf[:], ssum[:], 0.5)
                    nc.vector.tensor_scalar(
                        out=thr[:, bc * NM:(bc + 1) * NM], in0=expt[:],
                        scalar1=shalf[:], scalar2=1.0, op0=ALU.is_gt, op1=ALU.add,
                    )
                # select for this half (all on DVE; gpsimd runs these
                # strided ops 4-5x slower)
                cs = slice(bh * HBC, (bh + 1) * HBC)
                ws = slice(bh * HW_, (bh + 1) * HW_)
                for e in range(NE):
                    nc.vector.tensor_copy(threp_v[:, cs, :, e], thr_v[:, cs, :])
                nc.vector.tensor_scalar(
                    out=valrep[:, ws], in0=threp[:, ws], scalar1=-0.5,
                    scalar2=1.5, op0=ALU.mult, op1=ALU.add,
                )
                nc.vector.tensor_tensor(
                    outt[:, ws], acc[:, ws], threp[:, ws], ALU.is_lt
                )
                nc.vector.tensor_tensor(
                    outt[:, ws], outt[:, ws], valrep[:, ws], ALU.mult
                )
                # write this half's output now (half 0 hides under bh1 mms)
                nc.scalar.dma_start(o[:, ws], outt[:, ws])

            mm_block(0)
            tail_half(0)
            mm_block(1)
            tail_half(1)

    nc.compile()
    return nc


def _get_program():
    if "nc" not in _CACHE:
        _CACHE["nc"] = _build_program()
    return _CACHE["nc"]


def _split16(x):
    hi = x.astype(np.float16)
    lo = ((x - hi.astype(np.float32)) * SCALE).astype(np.float16)
    return hi, lo


def _const_input():
    base = (7.0 - np.arange(NE, dtype=np.float32))
    return np.ascontiguousarray(
        np.broadcast_to(np.tile(base, NBC * NM), (128, W))
    )


def _prep_core(i, ph, wg_full, rn):
    bsl = slice(i * BS, (i + 1) * BS)
    xc = np.ascontiguousarray(ph[bsl].T).reshape(NK, 128, BS)  # [32,128,1024]
    hi, lo = _split16(xc)
    # piece P = 8*bh + p: [128, cl(4) x limb(2) x 512]
    pieces = []
    for bh in (0, 1):
        hs = hi[:, :, bh * 512:(bh + 1) * 512]    # [32,128,512]
        ls = lo[:, :, bh * 512:(bh + 1) * 512]
        # [32,2,128,512] -> per piece p: chunks 4p..4p+3
        both = np.stack([hs, ls], axis=1)         # [32,2,128,512]
        arr = both.reshape(8, 4, 2, 128, 512).transpose(0, 3, 1, 2, 4)
        pieces.append(arr.reshape(8 * 128, 4096))
    xi = np.ascontiguousarray(np.concatenate(pieces, axis=0))  # [2048, 4096]

    esl = slice(i * SH, (i + 1) * SH)
    wgs = wg_full[esl]                            # [512, 4096] own rows
    wps = []
    for p in range(WPC):
        blk = wgs[:, p * 1024:(p + 1) * 1024]     # [512, 1024]
        bhi, blo = _split16(blk.reshape(NEC, 128, 1024))
        both = np.stack([bhi, blo], axis=1)       # [4,2,128,1024]
        wps.append(both.transpose(2, 0, 1, 3).reshape(128, 8192))
    wgi = np.ascontiguousarray(np.concatenate(wps, axis=0))    # [512, 8192]

    # nz[p, c*56 + m*8 + e] = rn[m, 1024*i + 128*c + p, e]
    nz_i = np.ascontiguousarray(
        rn[:, bsl, :].transpose(1, 0, 2)
        .reshape(NBC, 128, GRP).transpose(1, 0, 2).reshape(128, W)
    )

    # Wr slice for own rows: wrt[p, 7c+m] = Wr[m, 512i + 128c + p]
    wrt = np.ascontiguousarray(
        _WR_T[esl].reshape(NEC, 128, NM).transpose(1, 0, 2).reshape(128, NEC * NM)
    )
    wrh_i, wrl_i = _split16(wrt)
    return xi, wgi, nz_i, np.ascontiguousarray(wrh_i), np.ascontiguousarray(wrl_i)


_WR_T = None


def kernel(pooled_hidden, Wg, Wr, rand_noise):
    global LAST_RESULTS, _WR_T
    ph = np.asarray(pooled_hidden, dtype=np.float32)
    wg_full = np.asarray(Wg, dtype=np.float32)
    wr = np.asarray(Wr, dtype=np.float32)
    rn = np.asarray(rand_noise, dtype=np.float32)
    _WR_T = np.ascontiguousarray(wr.T)            # [4096, 7]

    nc = _get_program()
    cst = _const_input()

    in_maps = []
    for i in range(N_CORES):
        xi, wgi, nz_i, wrh_i, wrl_i = _prep_core(i, ph, wg_full, rn)
        in_maps.append(
            {"xi": xi, "wgi": wgi, "wrh": wrh_i, "wrl": wrl_i,
             "nz": nz_i, "cst": cst}
        )

    res = run_bass_kernel_spmd(nc, in_maps, list(range(N_CORES)))
    LAST_RESULTS = res

    out = np.empty((NM, B, NE), dtype=np.float32)
    for i, r in enumerate(res.results):
        oc = r["o"]  # [128, 448]
        out[:, i * BS:(i + 1) * BS, :] = (
            oc.reshape(128, NBC, NM, NE).transpose(2, 1, 0, 3).reshape(NM, BS, NE)
        )
    return out
